# revision 27
# baseline (speedup 1.0000x reference)
# Trainium2 Bass kernel for BasedCrossAttention (sparse_attention).
#
# Sharding: 8 cores = 2 batches x 4 head-groups (4 heads each).
# Each core computes, for its (batch, 4 heads):
#   rmsnorm(x) -> q / window-kv projections, encoder -> kv projections,
#   Taylor linear cross-attention (redundant F=273 feature map), sliding
#   window (64) causal self-attention, and a partial out-projection.
# Host: transposes inputs once, slices weights per core, sums the 4
# partial out-projections per batch and adds the residual.
#
# On-chip layout is "transposed" (feature-major): activations live as
# [d, t] with d on partitions so every matmul contracts over partitions.
#
# fp8 (e4m3) with DoubleRow perf mode is used on the k / q / kwin
# projections and the kv_state accumulation (2 K-chunks per matmul at
# 2 rows/cycle); the v paths and out-projection stay bf16 for accuracy.
import math
from contextlib import ExitStack

import ml_dtypes
import numpy as np

import concourse.bass as bass
import concourse.tile as tile
from concourse import bacc, mybir
from concourse.bass_utils import run_bass_kernel_spmd

F32 = mybir.dt.float32
BF = mybir.dt.bfloat16
F8 = mybir.dt.float8e4
AF = mybir.ActivationFunctionType
DR = mybir.MatmulPerfMode.DoubleRow
MUL = mybir.AluOpType.mult

D = 1024
NH = 16
HD = 64
FI = 16  # feature input dim
NQ = 136  # triu quadratic features
FTOT = NQ + FI + 1  # 153: [quad(136), lin(16), ones(1)]
C1 = FTOT - 128  # 25: second F chunk
WIN = 64
EPS_NORM = 1e-6
EPS_DEN = 1e-6
HPC = 4  # heads per core
DC = D // 128  # 8 d-model chunks
W8SC = 32.0  # host-side fp8 weight scale
PHSC = 16.0  # on-chip phi_k fp8 scale

PHIK_FP8 = True  # kv_state accumulation in fp8 DoubleRow
WARMUP = 30  # PE clock-ramp matmuls before real work
OUT_FP8 = True  # out-projection in fp8 DoubleRow (comb + Wout quantized)


def build_program(T=2048, debug=False):
    """One SPMD program; per-core variation comes only through inputs."""
    NB = T // 128  # 128-token blocks
    NI4 = T // 512  # 512-token chunks
    TH = T // 2  # half for attn/out chunking
    CW = 512  # free-dim chunk width
    assert T % 512 == 0

    nc = bacc.Bacc("TRN2", target_bir_lowering=False, debug=debug, num_devices=8)

    # ---- DRAM I/O ----
    xT8 = nc.dram_tensor("xT8", [D, T], F8, kind="ExternalInput")
    xTb = nc.dram_tensor("xTb", [D, T], BF, kind="ExternalInput")
    encT8 = nc.dram_tensor("encT8", [D, T], F8, kind="ExternalInput")
    Wq8 = nc.dram_tensor("Wq8", [D, HPC * HD], F8, kind="ExternalInput")
    Wk8 = nc.dram_tensor("Wk8", [D, HPC * HD], F8, kind="ExternalInput")
    Wwk8 = nc.dram_tensor("Wwk8", [D, HPC * HD], F8, kind="ExternalInput")
    Wv8 = nc.dram_tensor("Wv8", [D, HPC * HD], F8, kind="ExternalInput")
    Wwv = nc.dram_tensor("Wwv", [D, HPC * HD], BF, kind="ExternalInput")
    WqfA0 = nc.dram_tensor("WqfA0", [HD, 128], BF, kind="ExternalInput")
    WqfB0 = nc.dram_tensor("WqfB0", [HD, 128], BF, kind="ExternalInput")
    WqfC = nc.dram_tensor("WqfC", [128, 64], BF, kind="ExternalInput")
    WqfD = nc.dram_tensor("WqfD", [128, 40], BF, kind="ExternalInput")
    WkfAB = nc.dram_tensor("WkfAB", [HD, NQ + FI + NQ], BF, kind="ExternalInput")
    WoutA = nc.dram_tensor("WoutA", [HPC * 128, D], F8 if OUT_FP8 else BF,
                           kind="ExternalInput")
    mask_pack_d = nc.dram_tensor("mask_pack", [128, 512], BF, kind="ExternalInput")
    ident_d = nc.dram_tensor("ident", [128, 128], BF, kind="ExternalInput")
    out_d = nc.dram_tensor("out", [D, T], BF, kind="ExternalOutput")

    VDT = F8 if PHIK_FP8 else BF

    with tile.TileContext(nc) as tc, ExitStack() as ctx:
        persist = ctx.enter_context(tc.tile_pool(name="persist", bufs=1))

        def load_w(dram, shape, rearr=None):
            t = persist.tile(shape, dram.dtype, name=f"w_{dram.name}",
                             tag=f"w_{dram.name}")
            src = dram.ap() if rearr is None else dram.ap().rearrange(rearr, p=128)
            nc.sync.dma_start(out=t, in_=src)
            return t

        def load_w2(dram, n):
            # small [64, n] weight duplicated into both partition halves so it
            # can pair with operands at base_partition 0 or 64
            t = persist.tile([128, n], BF, name=f"w2_{dram.name}", tag=f"w2_{dram.name}")
            nc.sync.dma_start(out=t[0:64, :], in_=dram.ap())
            nc.sync.dma_start(out=t[64:128, :], in_=dram.ap())
            return t

        wk = load_w(Wk8, [128, DC, HPC * HD], "(c p) n -> p c n")
        wq = load_w(Wq8, [128, DC, HPC * HD], "(c p) n -> p c n")
        wwk = load_w(Wwk8, [128, DC, HPC * HD], "(c p) n -> p c n")
        wv = load_w(Wv8, [128, DC, HPC * HD], "(c p) n -> p c n")
        wwv = load_w(Wwv, [128, DC, HPC * HD], "(c p) n -> p c n")
        mask_pack = load_w(mask_pack_d, [128, 512])
        ident = load_w(ident_d, [128, 128])

        ones_col = persist.tile([128, 1], F32)
        ones_b = persist.tile([128, 1], BF)
        eps_t = persist.tile([1, 1], F32)

        # Long-lived activations
        kvs = [persist.tile([128, 130], BF, tag=f"kvs{h}", name=f"kvs{h}") for h in range(HPC)]
        kT = [persist.tile([128, T], BF, tag=f"kT{hp}", name=f"kT{hp}") for hp in range(2)]
        kwT = [persist.tile([128, T], BF, tag=f"kwT{hp}", name=f"kwT{hp}") for hp in range(2)]
        qT = [persist.tile([128, T], BF, tag=f"qT{hp}", name=f"qT{hp}") for hp in range(2)]
        vA = persist.tile([128, HPC, NB, 80], VDT, tag="vA", name="vA")
        vwa = persist.tile([128, HPC, NB, 65], BF, tag="vwa", name="vwa")
        rrT = persist.tile([128, NI4 * 4], F32, tag="rrT", name="rrT")  # rstd token-major
        phiq0 = [persist.tile([128, T], BF, tag=f"phiq0_{h}", name=f"phiq0_{h}")
                 for h in range(HPC)]
        phiq1 = [persist.tile([C1, T], BF, tag=f"phiq1_{h}", name=f"phiq1_{h}")
                 for h in range(HPC)]


        xT_r = xT8.ap().rearrange("(c p) t -> p c t", p=128)
        xTb_r = xTb.ap().rearrange("(c p) t -> p c t", p=128)
        encT_r = encT8.ap().rearrange("(c p) t -> p c t", p=128)

        # feature-map weights (used by feat chunks inside Phase 1)
        wqfA0 = load_w2(WqfA0, 128)
        wqfB0 = load_w2(WqfB0, 128)
        wqfC = load_w(WqfC, [128, 64])
        wqfD = load_w(WqfD, [128, 40])
        wkfAB = load_w2(WkfAB, NQ + FI + NQ)

        def emit_feat_chunk(sb, ps, hp, j2):
            # phi_q features for head pair (hp) over 512 tokens
            gsl = slice(j2 * CW, (j2 + 1) * CW)
            for u in range(2):
                h = 2 * hp + u
                ho = u * 64
                qtt = qT[hp][ho:ho + 64, gsl]
                p0 = ps.tile([128, CW], F32, tag="fb", bufs=2, name="p0")
                pb0 = ps.tile([128, CW], F32, tag="fb", bufs=2, name="pb0")
                nc.tensor.matmul(p0, (wqfA0[ho:ho + 64, :]), (qtt))
                nc.tensor.matmul(pb0, (wqfB0[ho:ho + 64, :]), (qtt))
                pb_sb = sb.tile([128, CW], BF, tag="pb_sb", bufs=3)
                if u == 0:
                    nc.scalar.copy(pb_sb, pb0)
                    nc.vector.tensor_mul(phiq0[h][:, gsl], p0, pb_sb)
                else:
                    p0_sb = sb.tile([128, CW], BF, tag="p0_sb", bufs=2)
                    nc.scalar.copy(p0_sb, p0)
                    nc.vector.tensor_copy(pb_sb, pb0)
                    nc.gpsimd.tensor_mul(phiq0[h][:, gsl], p0_sb, pb_sb)
            # paired heads: A1/B1 small features in one matmul each
            qtf = qT[hp][:, gsl]
            p1p = ps.tile([64, CW], F32, tag="fb", bufs=2, name="p1p")
            pb1p = ps.tile([40, CW], F32, tag="fb", bufs=2, name="pb1p")
            nc.tensor.matmul(p1p, (wqfC), (qtf))
            nc.tensor.matmul(pb1p, (wqfD), (qtf))
            for u in range(2):
                h = 2 * hp + u
                nc.scalar.copy(phiq1[h][0:C1 - 1, gsl], p1p[32 * u:32 * u + 24, :])
                nc.vector.tensor_mul(phiq1[h][0:8, gsl], phiq1[h][0:8, gsl],
                                     pb1p[32 * u:32 * u + 8, :])

        # ====== Phase 1: A1 (encoder kv) + B1 (x projections) + phi_q ======
        ctx1 = ExitStack()
        ctx1.enter_context(nc.named_scope("P1_proj"))
        p1 = ctx1.enter_context(tc.tile_pool(name="p1", bufs=2))
        p1ps = ctx1.enter_context(tc.tile_pool(name="p1ps", bufs=1, space="PSUM"))

        # warm the PE clock while the input DMAs land (uses the wk weights,
        # available right after the first weight DMA)
        ones8 = persist.tile([128, 2, 16], F8, name="ones8", tag="ones8")
        nc.vector.memset(ones8, 1.0)  # pair stride 16 for DR ldweights
        wps = p1ps.tile([128, 512], F32, tag="big", bufs=3)
        for _ in range(WARMUP):
            nc.tensor.matmul(wps, (wk[:, 0, 0:128]),
                             (wk[:, 0:2, :]), start=True, stop=True)
        nc.vector.memset(ones_col, 1.0)
        nc.vector.memset(ones_b, 1.0)
        nc.vector.memset(eps_t, EPS_NORM)
        # ones columns (col 64 of vA / vwa survives the 0:64 writes)
        nc.vector.memset(vA[:, :, :, 64:65], 1.0)  # pitch 80 for DR ldweights
        nc.gpsimd.memset(vwa[:, :, :, 64:65], 1.0)
        for h in range(HPC):
            nc.vector.memset(phiq1[h], 1.0)  # ones row (rest overwritten)

        for i4 in range(NI4):
            tsl = slice(i4 * 512, (i4 + 1) * 512)
            et8 = p1.tile([128, DC, 512], F8, tag="et8", bufs=3)
            nc.sync.dma_start(out=et8, in_=encT_r[:, :, tsl])
            xt8 = p1.tile([128, DC, 512], F8, tag="xt8", bufs=3)
            nc.sync.dma_start(out=xt8, in_=xT_r[:, :, tsl])
            xtb = p1.tile([128, DC, 512], BF, tag="xtb", bufs=3)
            nc.sync.dma_start(out=xtb, in_=xTb_r[:, :, tsl])

            # squares for rmsnorm stats (split engines), fp8 pair tiles
            sqp = []
            for cp in range(4):
                sqp.append(p1.tile([128, 2, 512], F8, tag=f"sq{cp}", bufs=2,
                                   name=f"sq{cp}"))
            for c in range(DC):
                sq = sqp[c // 2][:, c % 2, :]
                xc = xtb[:, c, :]
                if c < 3:
                    nc.scalar.square(sq, xc)
                elif c < 5:
                    nc.vector.tensor_mul(sq, xc, xc)
                else:
                    nc.gpsimd.tensor_mul(sq, xc, xc)

            # A1-k: fp8 DoubleRow over chunk pairs
            for hp in range(2):
                ps = p1ps.tile([128, 512], F32, tag="big", bufs=3)
                for cp in range(4):
                    nc.tensor.matmul(
                        ps, (wk[:, 2 * cp:2 * cp + 2, hp * 128:(hp + 1) * 128]),
                        (et8[:, 2 * cp:2 * cp + 2, :]),
                        start=(cp == 0), stop=(cp == 3), perf_mode=DR)
                if hp == 0:
                    nc.scalar.activation(kT[hp][:, tsl], ps, AF.Copy, scale=1.0 / W8SC)
                else:
                    nc.vector.tensor_scalar_mul(kT[hp][:, tsl], ps, 1.0 / W8SC)

            # sumsq via fp8 DoubleRow ones-matmul accumulation
            ssp = p1ps.tile([1, 512], F32, tag="ss", bufs=1)
            for cp in range(4):
                nc.tensor.matmul(ssp, ones8[:, :, 0:1], sqp[cp],
                                 start=(cp == 0), stop=(cp == 3), perf_mode=DR)
            sd = p1.tile([1, 512], F32, tag="sd")
            nc.scalar.activation(sd, ssp, AF.Sqrt, bias=eps_t[0:1, 0:1], scale=1.0 / D)
            rr = p1.tile([1, 512], F32, tag="rr")
            rr_scr = p1.tile([1, 512], F32, tag="rr_scr")
            nc.vector.reciprocal_approx_accurate(rr, sd, rr_scr)
            rstdB = p1.tile([128, 512], F32, tag="rstdB")
            nc.gpsimd.partition_broadcast(rstdB, rr)

            # B1 q / kwin: fp8 DoubleRow, column-scaled by rstd/32
            for w_sb, dst in ((wq, qT), (wwk, kwT)):
                for hp in range(2):
                    ps = p1ps.tile([128, 512], F32, tag="big", bufs=3)
                    for cp in range(4):
                        nc.tensor.matmul(
                            ps, (w_sb[:, 2 * cp:2 * cp + 2, hp * 128:(hp + 1) * 128]),
                            (xt8[:, 2 * cp:2 * cp + 2, :]),
                            start=(cp == 0), stop=(cp == 3), perf_mode=DR)
                    nc.vector.scalar_tensor_tensor(
                        dst[hp][:, tsl], ps, 1.0 / W8SC, rstdB, op0=MUL, op1=MUL)

            # A1-v: token-major v blocks (fp8 DoubleRow, kv-state path)
            for tb in range(4):
                blk = i4 * 4 + tb
                ps = p1ps.tile([128, HPC * HD], F32, tag="v", bufs=2)
                for cp in range(4):
                    nc.tensor.matmul(
                        ps, (et8[:, 2 * cp:2 * cp + 2, tb * 128:(tb + 1) * 128]),
                        (wv[:, 2 * cp:2 * cp + 2, :]),
                        start=(cp == 0), stop=(cp == 3), perf_mode=DR)
                if tb % 2 == 0:
                    nc.vector.tensor_scalar_mul(vA[:, :, blk, 0:HD], ps, 1.0 / W8SC)
                else:
                    nc.scalar.activation(vA[:, :, blk, 0:HD], ps, AF.Copy,
                                         scale=1.0 / W8SC)

            # token-major rstd (for vwin scaling): transpose via matmul
            for tb in range(4):
                rtp = p1ps.tile([128, 1], F32, tag="ss", bufs=1)
                nc.tensor.matmul(rtp, rr[0:1, tb * 128:(tb + 1) * 128],
                                 ones_col[0:1, 0:1])
                nc.vector.tensor_copy(rrT[:, i4 * 4 + tb:i4 * 4 + tb + 1], rtp)

            # B1 vwin: token-major, row-scaled by rstd (bf16)
            for tb in range(4):
                blk = i4 * 4 + tb
                ps = p1ps.tile([128, HPC * HD], F32, tag="v", bufs=2)
                for c in range(DC):
                    nc.tensor.matmul(
                        ps, (xtb[:, c, tb * 128:(tb + 1) * 128]),
                        (wwv[:, c, :]), start=(c == 0), stop=(c == DC - 1))
                nc.scalar.activation(vwa[:, :, blk, 0:HD], ps, AF.Copy,
                                     scale=rrT[:, blk:blk + 1])

            # phi_q features for this 512-token chunk (both head pairs)
            emit_feat_chunk(p1, p1ps, 0, i4)
            emit_feat_chunk(p1, p1ps, 1, i4)
        ctx1.close()

        # ========== Phase 2: A2 (phi_k features + kv_state) ==========
        ctx2 = ExitStack()
        ctx2.enter_context(nc.named_scope("P2_feat"))
        p2 = ctx2.enter_context(tc.tile_pool(name="p2", bufs=1))
        p2ps = ctx2.enter_context(tc.tile_pool(name="p2ps", bufs=1, space="PSUM"))

        # persistent phik ping-pong pair buffers [128, 2, 153]
        phbuf = [[p2.tile([128, 2, FTOT], VDT, tag=f"ph{u}{b}", name=f"ph{u}{b}")
                  for b in range(3)] for u in range(2)]
        for u in range(2):
            for b in range(3):
                nc.vector.memset(phbuf[u][b][:, :, FTOT - 1:FTOT], PHSC)

        for hp in range(2):
            kvt2 = [p2ps.tile([65, FTOT], F32, tag=f"kvt{u}", bufs=1,
                              name=f"kvt{u}") for u in range(2)]
            for tbp in range(NB // 2):
                repA = [p2ps.tile([128, 2, NQ + FI], F32, tag="repA", bufs=3,
                                  name=f"repA{u}") for u in range(2)]
                repB = [p2ps.tile([128, 2, NQ], F32, tag="repB", bufs=3,
                                  name=f"repB{u}") for u in range(2)]
                for s2 in range(2):
                    tb = 2 * tbp + s2
                    ts_ = slice(tb * 128, (tb + 1) * 128)
                    for u in range(2):
                        ho = u * 64
                        nc.tensor.matmul(repA[u][:, s2, :], (kT[hp][ho:ho + 64, ts_]),
                                         (wkfAB[ho:ho + 64, 0:NQ + FI]))
                        nc.tensor.matmul(repB[u][:, s2, :], (kT[hp][ho:ho + 64, ts_]),
                                         (wkfAB[ho:ho + 64, NQ + FI:]))
                for u in range(2):
                    h = 2 * hp + u
                    ph = phbuf[u][tbp % 3]
                    nc.scalar.activation(ph[:, :, 0:NQ + FI], repA[u],
                                         AF.Copy, scale=PHSC)
                    nc.vector.tensor_mul(ph[:, :, 0:NQ], ph[:, :, 0:NQ], repB[u])
                    if PHIK_FP8:
                        nc.tensor.matmul(kvt2[u], (vA[:, h, 2 * tbp:2 * tbp + 2, 0:65]),
                                         (ph), start=(tbp == 0),
                                         stop=(tbp == NB // 2 - 1), perf_mode=DR)
                    else:
                        for s2 in range(2):
                            nc.tensor.matmul(
                                kvt2[u], (vA[:, h, 2 * tbp + s2, 0:65]),
                                (ph[:, s2, :]),
                                start=(tbp == 0 and s2 == 0),
                                stop=(tbp == NB // 2 - 1 and s2 == 1))
            for u in range(2):
                h = 2 * hp + u
                # kv_state^T [65, 153] -> F-major kvs[h] via PE transposes
                kvt_sb = p2.tile([65, FTOT], BF, tag="kvt_sb", bufs=2)
                nc.scalar.activation(kvt_sb, kvt2[u], AF.Copy, scale=1.0 / PHSC)
                tp0 = p2ps.tile([128, 65], BF, tag="repA", bufs=3)
                nc.tensor.transpose(tp0, kvt_sb[:, 0:128], ident[0:65, 0:65])
                nc.vector.tensor_copy(kvs[h][:, 0:65], tp0)
                tp1 = p2ps.tile([C1, 65], BF, tag="repA", bufs=3)
                nc.tensor.transpose(tp1, kvt_sb[:, 128:FTOT], ident[0:65, 0:65])
                nc.vector.tensor_copy(kvs[h][0:C1, 65:130], tp1)
        ctx2.close()

        # ============ Phase 3: window attention + linear attn + out ========
        wout = load_w(WoutA, [128, HPC, D], "(h p) n -> p h n")
        with tc.tile_pool(name="p3", bufs=2) as p3, \
             tc.tile_pool(name="p3k", bufs=2) as p3k, \
             tc.tile_pool(name="p3ps", bufs=1, space="PSUM") as p3ps:
            for half in range(2):
                hof = half * TH
                CDT = F8 if OUT_FP8 else BF
                combT = [p3k.tile([128, 2, TH], CDT, tag=f"combT{hh}",
                                  name=f"combT{hh}") for hh in range(2)]

                # ---- attention (j outer so out-proj can interleave) ----
                for jl in range(TH // 256):
                    j = half * (TH // 256) + jl  # global superblock
                    for hp in range(2):
                        qsl = slice(j * 256, (j + 1) * 256)
                        qslA = slice(j * 256, j * 256 + 128)
                        qslB = slice(j * 256 + 128, (j + 1) * 256)
                        # packed scores [kbL q0:128 | kb0 q0:256 | kb1 q128:256],
                        # paired heads in disjoint PE row groups
                        sps = [p3ps.tile([128, 512], F32, tag="big", bufs=3,
                                         name=f"S{u}") for u in range(2)]
                        if j > 0:
                            for u in range(2):
                                ho = u * 64
                                nc.tensor.matmul(
                                    sps[u][:, 0:128],
                                    (kwT[hp][ho:ho + 64, (2 * j - 1) * 128:2 * j * 128]),
                                    (qT[hp][ho:ho + 64, qslA]))
                        for u in range(2):
                            ho = u * 64
                            nc.tensor.matmul(
                                sps[u][:, 128:384],
                                (kwT[hp][ho:ho + 64, 2 * j * 128:(2 * j + 1) * 128]),
                                (qT[hp][ho:ho + 64, qsl]))
                        for u in range(2):
                            ho = u * 64
                            nc.tensor.matmul(
                                sps[u][:, 384:512],
                                (kwT[hp][ho:ho + 64, (2 * j + 1) * 128:(2 * j + 2) * 128]),
                                (qT[hp][ho:ho + 64, qslB]))
                        exs = []
                        for u in range(2):
                            ex = p3.tile([128, 512], BF, tag=f"exps{u}", bufs=4,
                                         name=f"exps{u}")
                            eng = nc.vector
                            if j > 0:
                                nc.scalar.activation(ex, sps[u], AF.Exp, scale=0.125)
                                eng.tensor_mul(ex, ex, mask_pack)
                            else:
                                nc.scalar.activation(ex[:, 128:512], sps[u][:, 128:512],
                                                     AF.Exp, scale=0.125)
                                eng.tensor_mul(ex[:, 128:512], ex[:, 128:512],
                                               mask_pack[:, 128:512])
                            exs.append(ex)
                        for u in range(2):
                            h = 2 * hp + u
                            ex = exs[u]
                            for qh in range(2):  # two 128-q blocks in superblock
                                qb = 2 * j + qh
                                gq = slice(qb * 128, (qb + 1) * 128)  # phiq cols
                                lq = slice((qb * 128) - hof, (qb * 128) - hof + 128)
                                # linear path (cols 0:65) + window path (65:130)
                                lp = p3ps.tile([128, 130], F32, tag="lin", bufs=3)
                                nc.tensor.matmul(lp[:, 0:65], (phiq0[h][:, gq]),
                                                 (kvs[h][:, 0:65]), start=True, stop=False)
                                nc.tensor.matmul(lp[:, 0:65], (phiq1[h][:, gq]),
                                                 (kvs[h][0:C1, 65:130]), start=False, stop=True)
                                if qh == 0:
                                    pvs = ([] if j == 0 else [slice(0, 128)]) + [slice(128, 256)]
                                else:
                                    pvs = [slice(256, 384), slice(384, 512)]
                                kb0 = qb - 1 if (qh == 0 and j > 0) or qh == 1 else qb
                                for ki, exsl in enumerate(pvs):
                                    kb = kb0 + ki
                                    nc.tensor.matmul(lp[:, 65:130], (ex[:, exsl]),
                                                     (vwa[:, h, kb, :]),
                                                     start=(ki == 0), stop=(ki == len(pvs) - 1))
                                # one reciprocal for both normalizers (cols 64, 129)
                                rnl = p3.tile([128, 2], F32, tag="rnl", bufs=4)
                                nc.vector.reciprocal(rnl, lp[:, 64:130:65])
                                comb = p3.tile([128, 128], BF, tag="comb", bufs=4)
                                nc.vector.tensor_scalar_mul(comb[:, 0:64], lp[:, 0:64],
                                                            rnl[:, 0:1])
                                nc.vector.tensor_scalar_mul(comb[:, 64:128], lp[:, 65:129],
                                                            rnl[:, 1:2])
                                # transpose [tok, ch] -> [ch, tok]
                                ct = p3ps.tile([128, 128], BF, tag="ct", bufs=1)
                                nc.tensor.transpose(ct, comb, ident)
                                cdst = combT[h // 2][:, h % 2, lq]
                                if (u + qh) % 2 == 0:
                                    nc.scalar.copy(cdst, ct)
                                else:
                                    nc.vector.tensor_copy(cdst, ct)

                    # ---- out-projection chunk once its superblocks are done ----
                    if jl % 2 == 1:
                        j2 = jl // 2
                        lsl = slice(j2 * CW, (j2 + 1) * CW)
                        gsl = slice(hof + j2 * CW, hof + (j2 + 1) * CW)
                        for dc in range(DC):
                            po = p3ps.tile([128, CW], F32, tag="big", bufs=3, name="po")
                            if OUT_FP8:
                                for hh in range(2):
                                    nc.tensor.matmul(
                                        po, (wout[:, 2 * hh:2 * hh + 2,
                                                  dc * 128:(dc + 1) * 128]),
                                        (combT[hh][:, :, lsl]),
                                        start=(hh == 0), stop=(hh == 1),
                                        perf_mode=DR)
                            else:
                                for h in range(HPC):
                                    nc.tensor.matmul(
                                        po, (wout[:, h, dc * 128:(dc + 1) * 128]),
                                        (combT[h // 2][:, h % 2, lsl]),
                                        start=(h == 0), stop=(h == HPC - 1))
                            ob = p3.tile([128, CW], BF, tag="ob", bufs=3)
                            osc = 1.0 / W8SC if OUT_FP8 else 1.0
                            if dc % 2 == 0:
                                nc.scalar.activation(ob, po, AF.Copy, scale=osc)
                            else:
                                nc.vector.tensor_scalar_mul(ob, po, osc)
                            nc.sync.dma_start(
                                out=out_d.ap()[dc * 128:(dc + 1) * 128, gsl], in_=ob)
    nc.compile()
    return nc


# ---------------- host side ----------------

def _host_prep(x, encoder_out, norm_w, Wq, Wkv, Wqf, Wkf, Wwin, Wout, T):
    """Build the 8 per-core input maps."""
    bf = ml_dtypes.bfloat16
    f8 = ml_dtypes.float8_e4m3

    nw = norm_w.astype(np.float64)
    WqF = (nw[:, None] * Wq).astype(np.float32)
    WwinF = (nw[:, None] * Wwin).astype(np.float32)
    Wk_all, Wv_all = Wkv[:, :D], Wkv[:, D:]
    Wwk_all, Wwv_all = WwinF[:, :D], WwinF[:, D:]

    ti, tj = np.triu_indices(FI)
    sc = np.where(ti == tj, 0.5, 2.0 ** -0.5).astype(np.float64)
    WqfA_f = (sc * Wqf[:, ti]).astype(np.float32)  # [64, 136]
    WqfB_f = Wqf[:, tj]
    WkfA_f = (sc * Wkf[:, ti]).astype(np.float32)
    WkfB_f = Wkf[:, tj]
    WqfA0 = WqfA_f[:, :128]
    WqfA1 = np.concatenate([WqfA_f[:, 128:], Wqf], axis=1)       # [64, 24]
    WqfB0 = WqfB_f[:, :128]
    WqfB1 = np.ascontiguousarray(WqfB_f[:, 128:])                # [64, 8]
    # head-paired block-diagonal stationaries for the small feature matmuls
    WqfC = np.zeros((128, 64), np.float32)
    WqfC[0:64, 0:24] = WqfA1
    WqfC[64:128, 32:56] = WqfA1
    WqfD = np.zeros((128, 40), np.float32)
    WqfD[0:64, 0:8] = WqfB1
    WqfD[64:128, 32:40] = WqfB1
    WkfAB = np.concatenate([WkfA_f, Wkf, WkfB_f], axis=1)        # [64, 288]

    kq, qq = np.arange(128)[:, None], np.arange(256)[None, :]
    mask_mid = ((kq <= qq) & (kq >= qq - WIN)).astype(np.float32)
    qq1 = np.arange(128)[None, :]
    mask_left = (kq >= qq1 + WIN).astype(np.float32)
    # packed S layout: [kbL q''0:128 | kb0 q''0:256 | kb1 q''128:256]
    mask_pack = np.concatenate([mask_left, mask_mid, mask_mid[:, 0:128]], axis=1)
    ident = np.eye(128, dtype=np.float32)

    in_maps = []
    for c in range(8):
        b, g = c // 4, c % 4
        cols = slice(g * HPC * HD, (g + 1) * HPC * HD)
        WoutA = np.empty((HPC * 128, D), np.float32)
        for h in range(HPC):
            hg = g * HPC + h
            WoutA[h * 128:h * 128 + 64] = Wout[hg * 64:(hg + 1) * 64]
            WoutA[h * 128 + 64:(h + 1) * 128] = Wout[D + hg * 64:D + (hg + 1) * 64]
        xTc = np.ascontiguousarray(x[b, :T].T)
        encTc = np.ascontiguousarray(encoder_out[b, :T].T)
        in_maps.append({
            "xT8": xTc.astype(f8),
            "xTb": xTc.astype(bf),
            "encT8": encTc.astype(f8),
            "Wq8": np.ascontiguousarray(WqF[:, cols] * W8SC).astype(f8),
            "Wk8": np.ascontiguousarray(Wk_all[:, cols] * W8SC).astype(f8),
            "Wwk8": np.ascontiguousarray(Wwk_all[:, cols] * W8SC).astype(f8),
            "Wv8": np.ascontiguousarray(Wv_all[:, cols] * W8SC).astype(f8),
            "Wwv": np.ascontiguousarray(Wwv_all[:, cols]).astype(bf),
            "WqfA0": np.ascontiguousarray(WqfA0).astype(bf),
            "WqfB0": np.ascontiguousarray(WqfB0).astype(bf),
            "WqfC": WqfC.astype(bf),
            "WqfD": WqfD.astype(bf),
            "WkfAB": np.ascontiguousarray(WkfAB).astype(bf),
            "WoutA": ((WoutA * W8SC).astype(f8) if OUT_FP8
                      else WoutA.astype(bf)),
            "mask_pack": mask_pack.astype(bf),
            "ident": ident.astype(bf),
        })
    return in_maps


_BUILD_CACHE = {}


def run_sharded(inputs, T=2048, trace=False):
    if T not in _BUILD_CACHE:
        _BUILD_CACHE[T] = build_program(T=T)
    nc = _BUILD_CACHE[T]
    in_maps = _host_prep(T=T, **inputs)
    res = run_bass_kernel_spmd(nc, in_maps, core_ids=list(range(8)), trace=trace)
    x = inputs["x"]
    B = x.shape[0]
    out = np.array(x[:, :T], np.float32, copy=True)
    for c in range(8):
        out[c // 4] += res.results[c]["out"].T.astype(np.float32)
    return out, res


def kernel(**inputs):
    inputs = {k: np.asarray(v, np.float32) for k, v in inputs.items()}
    out, _ = run_sharded(inputs, T=2048, trace=False)
    return out


# revision 28
# speedup vs baseline: 1.0244x; 1.0244x over previous
# Trainium2 Bass kernel for BasedCrossAttention (sparse_attention).
#
# Sharding: 8 cores = 2 batches x 4 head-groups (4 heads each).
# Each core computes, for its (batch, 4 heads):
#   rmsnorm(x) -> q / window-kv projections, encoder -> kv projections,
#   Taylor linear cross-attention (redundant F=273 feature map), sliding
#   window (64) causal self-attention, and a partial out-projection.
# Host: transposes inputs once, slices weights per core, sums the 4
# partial out-projections per batch and adds the residual.
#
# On-chip layout is "transposed" (feature-major): activations live as
# [d, t] with d on partitions so every matmul contracts over partitions.
#
# fp8 (e4m3) with DoubleRow perf mode is used on the k / q / kwin
# projections and the kv_state accumulation (2 K-chunks per matmul at
# 2 rows/cycle); the v paths and out-projection stay bf16 for accuracy.
import math
from contextlib import ExitStack

import ml_dtypes
import numpy as np

import concourse.bass as bass
import concourse.tile as tile
from concourse import bacc, mybir
from concourse.bass_utils import run_bass_kernel_spmd

F32 = mybir.dt.float32
BF = mybir.dt.bfloat16
F8 = mybir.dt.float8e4
AF = mybir.ActivationFunctionType
DR = mybir.MatmulPerfMode.DoubleRow
MUL = mybir.AluOpType.mult

D = 1024
NH = 16
HD = 64
FI = 16  # feature input dim
NQ = 136  # triu quadratic features
FTOT = NQ + FI + 1  # 153: [quad(136), lin(16), ones(1)]
C1 = FTOT - 128  # 25: second F chunk
WIN = 64
EPS_NORM = 1e-6
EPS_DEN = 1e-6
HPC = 4  # heads per core
DC = D // 128  # 8 d-model chunks
W8SC = 32.0  # host-side fp8 weight scale
PHSC = 16.0  # on-chip phi_k fp8 scale

PHIK_FP8 = True  # kv_state accumulation in fp8 DoubleRow
WARMUP = 30  # PE clock-ramp matmuls before real work
OUT_FP8 = True  # out-projection in fp8 DoubleRow (comb + Wout quantized)


def build_program(T=2048, debug=False):
    """One SPMD program; per-core variation comes only through inputs."""
    NB = T // 128  # 128-token blocks
    NI4 = T // 512  # 512-token chunks
    TH = T // 2  # half for attn/out chunking
    CW = 512  # free-dim chunk width
    assert T % 512 == 0

    nc = bacc.Bacc("TRN2", target_bir_lowering=False, debug=debug, num_devices=8)

    # ---- DRAM I/O ----
    xT8 = nc.dram_tensor("xT8", [D, T], F8, kind="ExternalInput")
    xTb = nc.dram_tensor("xTb", [D, T], BF, kind="ExternalInput")
    encT8 = nc.dram_tensor("encT8", [D, T], F8, kind="ExternalInput")
    Wq8 = nc.dram_tensor("Wq8", [D, HPC * HD], F8, kind="ExternalInput")
    Wk8 = nc.dram_tensor("Wk8", [D, HPC * HD], F8, kind="ExternalInput")
    Wwk8 = nc.dram_tensor("Wwk8", [D, HPC * HD], F8, kind="ExternalInput")
    Wv8 = nc.dram_tensor("Wv8", [D, HPC * HD], F8, kind="ExternalInput")
    Wwv = nc.dram_tensor("Wwv", [D, HPC * HD], BF, kind="ExternalInput")
    WqfA0 = nc.dram_tensor("WqfA0", [HD, 128], BF, kind="ExternalInput")
    WqfB0 = nc.dram_tensor("WqfB0", [HD, 128], BF, kind="ExternalInput")
    WqfC = nc.dram_tensor("WqfC", [128, 64], BF, kind="ExternalInput")
    WqfD = nc.dram_tensor("WqfD", [128, 40], BF, kind="ExternalInput")
    WkfAB = nc.dram_tensor("WkfAB", [HD, NQ + FI + NQ], BF, kind="ExternalInput")
    WoutA = nc.dram_tensor("WoutA", [HPC * 128, D], F8 if OUT_FP8 else BF,
                           kind="ExternalInput")
    mask_pack_d = nc.dram_tensor("mask_pack", [128, 512], BF, kind="ExternalInput")
    ident_d = nc.dram_tensor("ident", [128, 128], BF, kind="ExternalInput")
    out_d = nc.dram_tensor("out", [D, T], BF, kind="ExternalOutput")

    VDT = F8 if PHIK_FP8 else BF

    with tile.TileContext(nc) as tc, ExitStack() as ctx:
        persist = ctx.enter_context(tc.tile_pool(name="persist", bufs=1))

        def load_w(dram, shape, rearr=None):
            t = persist.tile(shape, dram.dtype, name=f"w_{dram.name}",
                             tag=f"w_{dram.name}")
            src = dram.ap() if rearr is None else dram.ap().rearrange(rearr, p=128)
            nc.sync.dma_start(out=t, in_=src)
            return t

        def load_w2(dram, n):
            # small [64, n] weight duplicated into both partition halves so it
            # can pair with operands at base_partition 0 or 64
            t = persist.tile([128, n], BF, name=f"w2_{dram.name}", tag=f"w2_{dram.name}")
            nc.sync.dma_start(out=t[0:64, :], in_=dram.ap())
            nc.sync.dma_start(out=t[64:128, :], in_=dram.ap())
            return t

        wk = load_w(Wk8, [128, DC, HPC * HD], "(c p) n -> p c n")
        wq = load_w(Wq8, [128, DC, HPC * HD], "(c p) n -> p c n")
        wwk = load_w(Wwk8, [128, DC, HPC * HD], "(c p) n -> p c n")
        wv = load_w(Wv8, [128, DC, HPC * HD], "(c p) n -> p c n")
        wwv = load_w(Wwv, [128, DC, HPC * HD], "(c p) n -> p c n")
        mask_pack = load_w(mask_pack_d, [128, 512])
        ident = load_w(ident_d, [128, 128])

        ones_col = persist.tile([128, 1], F32)
        ones_b = persist.tile([128, 1], BF)
        eps_t = persist.tile([1, 1], F32)

        # Long-lived activations
        kvs = [persist.tile([128, 130], BF, tag=f"kvs{h}", name=f"kvs{h}") for h in range(HPC)]
        kT = [persist.tile([128, T], BF, tag=f"kT{hp}", name=f"kT{hp}") for hp in range(2)]
        kwT = [persist.tile([128, T], BF, tag=f"kwT{hp}", name=f"kwT{hp}") for hp in range(2)]
        qT = [persist.tile([128, T], BF, tag=f"qT{hp}", name=f"qT{hp}") for hp in range(2)]
        vA = persist.tile([128, HPC, NB, 80], VDT, tag="vA", name="vA")
        vwa = persist.tile([128, HPC, NB, 65], BF, tag="vwa", name="vwa")
        rrT = persist.tile([128, NI4 * 4], F32, tag="rrT", name="rrT")  # rstd token-major
        phiq0 = [persist.tile([128, T], BF, tag=f"phiq0_{h}", name=f"phiq0_{h}")
                 for h in range(HPC)]
        phiq1 = [persist.tile([C1, T], BF, tag=f"phiq1_{h}", name=f"phiq1_{h}")
                 for h in range(HPC)]


        xT_r = xT8.ap().rearrange("(c p) t -> p c t", p=128)
        xTb_r = xTb.ap().rearrange("(c p) t -> p c t", p=128)
        encT_r = encT8.ap().rearrange("(c p) t -> p c t", p=128)

        # feature-map weights (used by feat chunks inside Phase 1)
        wqfA0 = load_w2(WqfA0, 128)
        wqfB0 = load_w2(WqfB0, 128)
        wqfC = load_w(WqfC, [128, 64])
        wqfD = load_w(WqfD, [128, 40])
        wkfAB = load_w2(WkfAB, NQ + FI + NQ)

        def emit_feat_chunk(sb, ps, hp, j2):
            # phi_q features for head pair (hp) over 512 tokens
            gsl = slice(j2 * CW, (j2 + 1) * CW)
            for u in range(2):
                h = 2 * hp + u
                ho = u * 64
                qtt = qT[hp][ho:ho + 64, gsl]
                p0 = ps.tile([128, CW], F32, tag="fb", bufs=2, name="p0")
                pb0 = ps.tile([128, CW], F32, tag="fb", bufs=2, name="pb0")
                nc.tensor.matmul(p0, (wqfA0[ho:ho + 64, :]), (qtt))
                nc.tensor.matmul(pb0, (wqfB0[ho:ho + 64, :]), (qtt))
                pb_sb = sb.tile([128, CW], BF, tag="pb_sb", bufs=3)
                if u == 0:
                    nc.scalar.copy(pb_sb, pb0)
                    nc.vector.tensor_mul(phiq0[h][:, gsl], p0, pb_sb)
                else:
                    p0_sb = sb.tile([128, CW], BF, tag="p0_sb", bufs=2)
                    nc.scalar.copy(p0_sb, p0)
                    nc.vector.tensor_copy(pb_sb, pb0)
                    nc.gpsimd.tensor_mul(phiq0[h][:, gsl], p0_sb, pb_sb)
            # paired heads: A1/B1 small features in one matmul each
            qtf = qT[hp][:, gsl]
            p1p = ps.tile([64, CW], F32, tag="fb", bufs=2, name="p1p")
            pb1p = ps.tile([40, CW], F32, tag="fb", bufs=2, name="pb1p")
            nc.tensor.matmul(p1p, (wqfC), (qtf))
            nc.tensor.matmul(pb1p, (wqfD), (qtf))
            for u in range(2):
                h = 2 * hp + u
                nc.scalar.copy(phiq1[h][0:C1 - 1, gsl], p1p[32 * u:32 * u + 24, :])
                nc.vector.tensor_mul(phiq1[h][0:8, gsl], phiq1[h][0:8, gsl],
                                     pb1p[32 * u:32 * u + 8, :])

        # ====== Phase 1: A1 (encoder kv) + B1 (x projections) + phi_q ======
        ctx1 = ExitStack()
        ctx1.enter_context(nc.named_scope("P1_proj"))
        p1 = ctx1.enter_context(tc.tile_pool(name="p1", bufs=2))
        p1ps = ctx1.enter_context(tc.tile_pool(name="p1ps", bufs=1, space="PSUM"))

        # warm the PE clock while the input DMAs land (uses the wk weights,
        # available right after the first weight DMA)
        ones8 = persist.tile([128, 2, 16], F8, name="ones8", tag="ones8")
        nc.vector.memset(ones8, 1.0)  # pair stride 16 for DR ldweights
        wps = p1ps.tile([128, 512], F32, tag="big", bufs=3)
        for _ in range(WARMUP):
            nc.tensor.matmul(wps, (wk[:, 0, 0:128]),
                             (wk[:, 0:2, :]), start=True, stop=True)
        nc.vector.memset(ones_col, 1.0)
        nc.vector.memset(ones_b, 1.0)
        nc.vector.memset(eps_t, EPS_NORM)
        # ones columns (col 64 of vA / vwa survives the 0:64 writes)
        nc.vector.memset(vA[:, :, :, 64:65], 1.0)  # pitch 80 for DR ldweights
        nc.gpsimd.memset(vwa[:, :, :, 64:65], 1.0)
        for h in range(HPC):
            nc.vector.memset(phiq1[h], 1.0)  # ones row (rest overwritten)

        for i4 in range(NI4):
            tsl = slice(i4 * 512, (i4 + 1) * 512)
            et8 = p1.tile([128, DC, 512], F8, tag="et8", bufs=3)
            nc.sync.dma_start(out=et8, in_=encT_r[:, :, tsl])
            xt8 = p1.tile([128, DC, 512], F8, tag="xt8", bufs=3)
            nc.sync.dma_start(out=xt8, in_=xT_r[:, :, tsl])
            xtb = p1.tile([128, DC, 512], BF, tag="xtb", bufs=3)
            nc.sync.dma_start(out=xtb, in_=xTb_r[:, :, tsl])

            # squares for rmsnorm stats (split engines), fp8 pair tiles
            sqp = []
            for cp in range(4):
                sqp.append(p1.tile([128, 2, 512], F8, tag=f"sq{cp}", bufs=2,
                                   name=f"sq{cp}"))
            for c in range(DC):
                sq = sqp[c // 2][:, c % 2, :]
                xc = xtb[:, c, :]
                if c < 3:
                    nc.scalar.square(sq, xc)
                elif c < 5:
                    nc.vector.tensor_mul(sq, xc, xc)
                else:
                    nc.gpsimd.tensor_mul(sq, xc, xc)

            # A1-k: fp8 DoubleRow over chunk pairs
            for hp in range(2):
                ps = p1ps.tile([128, 512], F32, tag="big", bufs=3)
                for cp in range(4):
                    nc.tensor.matmul(
                        ps, (wk[:, 2 * cp:2 * cp + 2, hp * 128:(hp + 1) * 128]),
                        (et8[:, 2 * cp:2 * cp + 2, :]),
                        start=(cp == 0), stop=(cp == 3), perf_mode=DR)
                if hp == 0:
                    nc.scalar.activation(kT[hp][:, tsl], ps, AF.Copy, scale=1.0 / W8SC)
                else:
                    nc.vector.tensor_scalar_mul(kT[hp][:, tsl], ps, 1.0 / W8SC)

            # sumsq via fp8 DoubleRow ones-matmul accumulation
            ssp = p1ps.tile([1, 512], F32, tag="ss", bufs=1)
            for cp in range(4):
                nc.tensor.matmul(ssp, ones8[:, :, 0:1], sqp[cp],
                                 start=(cp == 0), stop=(cp == 3), perf_mode=DR)
            sd = p1.tile([1, 512], F32, tag="sd")
            nc.scalar.activation(sd, ssp, AF.Sqrt, bias=eps_t[0:1, 0:1], scale=1.0 / D)
            rr = p1.tile([1, 512], F32, tag="rr")
            rr_scr = p1.tile([1, 512], F32, tag="rr_scr")
            nc.vector.reciprocal_approx_accurate(rr, sd, rr_scr)
            rstdB = p1.tile([128, 512], F32, tag="rstdB")
            nc.gpsimd.partition_broadcast(rstdB, rr)

            # A1-v: token-major v blocks (fp8 DoubleRow, kv-state path)
            for tb in range(4):
                blk = i4 * 4 + tb
                ps = p1ps.tile([128, HPC * HD], F32, tag="v", bufs=2)
                for cp in range(4):
                    nc.tensor.matmul(
                        ps, (et8[:, 2 * cp:2 * cp + 2, tb * 128:(tb + 1) * 128]),
                        (wv[:, 2 * cp:2 * cp + 2, :]),
                        start=(cp == 0), stop=(cp == 3), perf_mode=DR)
                if tb % 2 == 0:
                    nc.vector.tensor_scalar_mul(vA[:, :, blk, 0:HD], ps, 1.0 / W8SC)
                else:
                    nc.scalar.activation(vA[:, :, blk, 0:HD], ps, AF.Copy,
                                         scale=1.0 / W8SC)

            # B1 q / kwin: fp8 DoubleRow, column-scaled by rstd/32
            for w_sb, dst in ((wq, qT), (wwk, kwT)):
                for hp in range(2):
                    ps = p1ps.tile([128, 512], F32, tag="big", bufs=3)
                    for cp in range(4):
                        nc.tensor.matmul(
                            ps, (w_sb[:, 2 * cp:2 * cp + 2, hp * 128:(hp + 1) * 128]),
                            (xt8[:, 2 * cp:2 * cp + 2, :]),
                            start=(cp == 0), stop=(cp == 3), perf_mode=DR)
                    nc.vector.scalar_tensor_tensor(
                        dst[hp][:, tsl], ps, 1.0 / W8SC, rstdB, op0=MUL, op1=MUL)

            # token-major rstd (for vwin scaling): transpose via matmul
            for tb in range(4):
                rtp = p1ps.tile([128, 1], F32, tag="ss", bufs=1)
                nc.tensor.matmul(rtp, rr[0:1, tb * 128:(tb + 1) * 128],
                                 ones_col[0:1, 0:1])
                nc.vector.tensor_copy(rrT[:, i4 * 4 + tb:i4 * 4 + tb + 1], rtp)

            # B1 vwin: token-major, row-scaled by rstd (bf16)
            for tb in range(4):
                blk = i4 * 4 + tb
                ps = p1ps.tile([128, HPC * HD], F32, tag="v", bufs=2)
                for c in range(DC):
                    nc.tensor.matmul(
                        ps, (xtb[:, c, tb * 128:(tb + 1) * 128]),
                        (wwv[:, c, :]), start=(c == 0), stop=(c == DC - 1))
                nc.scalar.activation(vwa[:, :, blk, 0:HD], ps, AF.Copy,
                                     scale=rrT[:, blk:blk + 1])

            # phi_q features for this 512-token chunk (both head pairs)
            emit_feat_chunk(p1, p1ps, 0, i4)
            emit_feat_chunk(p1, p1ps, 1, i4)
        ctx1.close()

        # ========== Phase 2: A2 (phi_k features + kv_state) ==========
        ctx2 = ExitStack()
        ctx2.enter_context(nc.named_scope("P2_feat"))
        p2 = ctx2.enter_context(tc.tile_pool(name="p2", bufs=1))
        p2ps = ctx2.enter_context(tc.tile_pool(name="p2ps", bufs=1, space="PSUM"))

        # persistent phik ping-pong pair buffers [128, 2, 153]
        phbuf = [[p2.tile([128, 2, FTOT], VDT, tag=f"ph{u}{b}", name=f"ph{u}{b}")
                  for b in range(3)] for u in range(2)]
        for u in range(2):
            for b in range(3):
                nc.vector.memset(phbuf[u][b][:, :, FTOT - 1:FTOT], PHSC)

        for hp in range(2):
            kvt2 = [p2ps.tile([65, FTOT], F32, tag=f"kvt{u}", bufs=1,
                              name=f"kvt{u}") for u in range(2)]
            for tbp in range(NB // 2):
                repA = [p2ps.tile([128, 2, NQ + FI], F32, tag="repA", bufs=3,
                                  name=f"repA{u}") for u in range(2)]
                repB = [p2ps.tile([128, 2, NQ], F32, tag="repB", bufs=3,
                                  name=f"repB{u}") for u in range(2)]
                for s2 in range(2):
                    tb = 2 * tbp + s2
                    ts_ = slice(tb * 128, (tb + 1) * 128)
                    for u in range(2):
                        ho = u * 64
                        nc.tensor.matmul(repA[u][:, s2, :], (kT[hp][ho:ho + 64, ts_]),
                                         (wkfAB[ho:ho + 64, 0:NQ + FI]))
                        nc.tensor.matmul(repB[u][:, s2, :], (kT[hp][ho:ho + 64, ts_]),
                                         (wkfAB[ho:ho + 64, NQ + FI:]))
                for u in range(2):
                    h = 2 * hp + u
                    ph = phbuf[u][tbp % 3]
                    nc.scalar.activation(ph[:, :, 0:NQ + FI], repA[u],
                                         AF.Copy, scale=PHSC)
                    nc.vector.tensor_mul(ph[:, :, 0:NQ], ph[:, :, 0:NQ], repB[u])
                    if PHIK_FP8:
                        nc.tensor.matmul(kvt2[u], (vA[:, h, 2 * tbp:2 * tbp + 2, 0:65]),
                                         (ph), start=(tbp == 0),
                                         stop=(tbp == NB // 2 - 1), perf_mode=DR)
                    else:
                        for s2 in range(2):
                            nc.tensor.matmul(
                                kvt2[u], (vA[:, h, 2 * tbp + s2, 0:65]),
                                (ph[:, s2, :]),
                                start=(tbp == 0 and s2 == 0),
                                stop=(tbp == NB // 2 - 1 and s2 == 1))
            for u in range(2):
                h = 2 * hp + u
                # kv_state^T [65, 153] -> F-major kvs[h] via PE transposes
                kvt_sb = p2.tile([65, FTOT], BF, tag="kvt_sb", bufs=2)
                nc.scalar.activation(kvt_sb, kvt2[u], AF.Copy, scale=1.0 / PHSC)
                tp0 = p2ps.tile([128, 65], BF, tag="repA", bufs=3)
                nc.tensor.transpose(tp0, kvt_sb[:, 0:128], ident[0:65, 0:65])
                nc.vector.tensor_copy(kvs[h][:, 0:65], tp0)
                tp1 = p2ps.tile([C1, 65], BF, tag="repA", bufs=3)
                nc.tensor.transpose(tp1, kvt_sb[:, 128:FTOT], ident[0:65, 0:65])
                nc.vector.tensor_copy(kvs[h][0:C1, 65:130], tp1)
        ctx2.close()

        # ============ Phase 3: window attention + linear attn + out ========
        wout = load_w(WoutA, [128, HPC, D], "(h p) n -> p h n")
        with tc.tile_pool(name="p3", bufs=2) as p3, \
             tc.tile_pool(name="p3k", bufs=2) as p3k, \
             tc.tile_pool(name="p3ps", bufs=1, space="PSUM") as p3ps:
            for half in range(2):
                hof = half * TH
                CDT = F8 if OUT_FP8 else BF
                combT = [p3k.tile([128, 2, TH], CDT, tag=f"combT{hh}",
                                  name=f"combT{hh}") for hh in range(2)]

                # ---- attention (j outer so out-proj can interleave) ----
                for jl in range(TH // 256):
                    j = half * (TH // 256) + jl  # global superblock
                    for hp in range(2):
                        qsl = slice(j * 256, (j + 1) * 256)
                        qslA = slice(j * 256, j * 256 + 128)
                        qslB = slice(j * 256 + 128, (j + 1) * 256)
                        # packed scores [kbL q0:128 | kb0 q0:256 | kb1 q128:256],
                        # paired heads in disjoint PE row groups
                        sps = [p3ps.tile([128, 512], F32, tag="big", bufs=3,
                                         name=f"S{u}") for u in range(2)]
                        if j > 0:
                            for u in range(2):
                                ho = u * 64
                                nc.tensor.matmul(
                                    sps[u][:, 0:128],
                                    (kwT[hp][ho:ho + 64, (2 * j - 1) * 128:2 * j * 128]),
                                    (qT[hp][ho:ho + 64, qslA]))
                        for u in range(2):
                            ho = u * 64
                            nc.tensor.matmul(
                                sps[u][:, 128:384],
                                (kwT[hp][ho:ho + 64, 2 * j * 128:(2 * j + 1) * 128]),
                                (qT[hp][ho:ho + 64, qsl]))
                        for u in range(2):
                            ho = u * 64
                            nc.tensor.matmul(
                                sps[u][:, 384:512],
                                (kwT[hp][ho:ho + 64, (2 * j + 1) * 128:(2 * j + 2) * 128]),
                                (qT[hp][ho:ho + 64, qslB]))
                        exs = []
                        for u in range(2):
                            ex = p3.tile([128, 512], BF, tag=f"exps{u}", bufs=4,
                                         name=f"exps{u}")
                            eng = nc.vector
                            if j > 0:
                                nc.scalar.activation(ex, sps[u], AF.Exp, scale=0.125)
                                eng.tensor_mul(ex, ex, mask_pack)
                            else:
                                nc.scalar.activation(ex[:, 128:512], sps[u][:, 128:512],
                                                     AF.Exp, scale=0.125)
                                eng.tensor_mul(ex[:, 128:512], ex[:, 128:512],
                                               mask_pack[:, 128:512])
                            exs.append(ex)
                        for u in range(2):
                            h = 2 * hp + u
                            ex = exs[u]
                            for qh in range(2):  # two 128-q blocks in superblock
                                qb = 2 * j + qh
                                gq = slice(qb * 128, (qb + 1) * 128)  # phiq cols
                                lq = slice((qb * 128) - hof, (qb * 128) - hof + 128)
                                # linear path (cols 0:65) + window path (65:130)
                                lp = p3ps.tile([128, 130], F32, tag="lin", bufs=3)
                                nc.tensor.matmul(lp[:, 0:65], (phiq0[h][:, gq]),
                                                 (kvs[h][:, 0:65]), start=True, stop=False)
                                nc.tensor.matmul(lp[:, 0:65], (phiq1[h][:, gq]),
                                                 (kvs[h][0:C1, 65:130]), start=False, stop=True)
                                if qh == 0:
                                    pvs = ([] if j == 0 else [slice(0, 128)]) + [slice(128, 256)]
                                else:
                                    pvs = [slice(256, 384), slice(384, 512)]
                                kb0 = qb - 1 if (qh == 0 and j > 0) or qh == 1 else qb
                                for ki, exsl in enumerate(pvs):
                                    kb = kb0 + ki
                                    nc.tensor.matmul(lp[:, 65:130], (ex[:, exsl]),
                                                     (vwa[:, h, kb, :]),
                                                     start=(ki == 0), stop=(ki == len(pvs) - 1))
                                # one reciprocal for both normalizers (cols 64, 129)
                                rnl = p3.tile([128, 2], F32, tag="rnl", bufs=4)
                                nc.vector.reciprocal(rnl, lp[:, 64:130:65])
                                comb = p3.tile([128, 128], BF, tag="comb", bufs=4)
                                nc.vector.tensor_scalar_mul(comb[:, 0:64], lp[:, 0:64],
                                                            rnl[:, 0:1])
                                nc.vector.tensor_scalar_mul(comb[:, 64:128], lp[:, 65:129],
                                                            rnl[:, 1:2])
                                # transpose [tok, ch] -> [ch, tok]
                                ct = p3ps.tile([128, 128], BF, tag="ct", bufs=1)
                                nc.tensor.transpose(ct, comb, ident)
                                cdst = combT[h // 2][:, h % 2, lq]
                                if (u + qh) % 2 == 0:
                                    nc.scalar.copy(cdst, ct)
                                else:
                                    nc.vector.tensor_copy(cdst, ct)

                    # ---- out-projection chunk once its superblocks are done ----
                    if jl % 2 == 1:
                        j2 = jl // 2
                        lsl = slice(j2 * CW, (j2 + 1) * CW)
                        gsl = slice(hof + j2 * CW, hof + (j2 + 1) * CW)
                        for dc in range(DC):
                            po = p3ps.tile([128, CW], F32, tag="big", bufs=3, name="po")
                            if OUT_FP8:
                                for hh in range(2):
                                    nc.tensor.matmul(
                                        po, (wout[:, 2 * hh:2 * hh + 2,
                                                  dc * 128:(dc + 1) * 128]),
                                        (combT[hh][:, :, lsl]),
                                        start=(hh == 0), stop=(hh == 1),
                                        perf_mode=DR)
                            else:
                                for h in range(HPC):
                                    nc.tensor.matmul(
                                        po, (wout[:, h, dc * 128:(dc + 1) * 128]),
                                        (combT[h // 2][:, h % 2, lsl]),
                                        start=(h == 0), stop=(h == HPC - 1))
                            ob = p3.tile([128, CW], BF, tag="ob", bufs=3)
                            osc = 1.0 / W8SC if OUT_FP8 else 1.0
                            if dc % 2 == 0:
                                nc.scalar.activation(ob, po, AF.Copy, scale=osc)
                            else:
                                nc.vector.tensor_scalar_mul(ob, po, osc)
                            nc.sync.dma_start(
                                out=out_d.ap()[dc * 128:(dc + 1) * 128, gsl], in_=ob)
    nc.compile()
    return nc


# ---------------- host side ----------------

def _host_prep(x, encoder_out, norm_w, Wq, Wkv, Wqf, Wkf, Wwin, Wout, T):
    """Build the 8 per-core input maps."""
    bf = ml_dtypes.bfloat16
    f8 = ml_dtypes.float8_e4m3

    nw = norm_w.astype(np.float64)
    WqF = (nw[:, None] * Wq).astype(np.float32)
    WwinF = (nw[:, None] * Wwin).astype(np.float32)
    Wk_all, Wv_all = Wkv[:, :D], Wkv[:, D:]
    Wwk_all, Wwv_all = WwinF[:, :D], WwinF[:, D:]

    ti, tj = np.triu_indices(FI)
    sc = np.where(ti == tj, 0.5, 2.0 ** -0.5).astype(np.float64)
    WqfA_f = (sc * Wqf[:, ti]).astype(np.float32)  # [64, 136]
    WqfB_f = Wqf[:, tj]
    WkfA_f = (sc * Wkf[:, ti]).astype(np.float32)
    WkfB_f = Wkf[:, tj]
    WqfA0 = WqfA_f[:, :128]
    WqfA1 = np.concatenate([WqfA_f[:, 128:], Wqf], axis=1)       # [64, 24]
    WqfB0 = WqfB_f[:, :128]
    WqfB1 = np.ascontiguousarray(WqfB_f[:, 128:])                # [64, 8]
    # head-paired block-diagonal stationaries for the small feature matmuls
    WqfC = np.zeros((128, 64), np.float32)
    WqfC[0:64, 0:24] = WqfA1
    WqfC[64:128, 32:56] = WqfA1
    WqfD = np.zeros((128, 40), np.float32)
    WqfD[0:64, 0:8] = WqfB1
    WqfD[64:128, 32:40] = WqfB1
    WkfAB = np.concatenate([WkfA_f, Wkf, WkfB_f], axis=1)        # [64, 288]

    kq, qq = np.arange(128)[:, None], np.arange(256)[None, :]
    mask_mid = ((kq <= qq) & (kq >= qq - WIN)).astype(np.float32)
    qq1 = np.arange(128)[None, :]
    mask_left = (kq >= qq1 + WIN).astype(np.float32)
    # packed S layout: [kbL q''0:128 | kb0 q''0:256 | kb1 q''128:256]
    mask_pack = np.concatenate([mask_left, mask_mid, mask_mid[:, 0:128]], axis=1)
    ident = np.eye(128, dtype=np.float32)

    in_maps = []
    for c in range(8):
        b, g = c // 4, c % 4
        cols = slice(g * HPC * HD, (g + 1) * HPC * HD)
        WoutA = np.empty((HPC * 128, D), np.float32)
        for h in range(HPC):
            hg = g * HPC + h
            WoutA[h * 128:h * 128 + 64] = Wout[hg * 64:(hg + 1) * 64]
            WoutA[h * 128 + 64:(h + 1) * 128] = Wout[D + hg * 64:D + (hg + 1) * 64]
        xTc = np.ascontiguousarray(x[b, :T].T)
        encTc = np.ascontiguousarray(encoder_out[b, :T].T)
        in_maps.append({
            "xT8": xTc.astype(f8),
            "xTb": xTc.astype(bf),
            "encT8": encTc.astype(f8),
            "Wq8": np.ascontiguousarray(WqF[:, cols] * W8SC).astype(f8),
            "Wk8": np.ascontiguousarray(Wk_all[:, cols] * W8SC).astype(f8),
            "Wwk8": np.ascontiguousarray(Wwk_all[:, cols] * W8SC).astype(f8),
            "Wv8": np.ascontiguousarray(Wv_all[:, cols] * W8SC).astype(f8),
            "Wwv": np.ascontiguousarray(Wwv_all[:, cols]).astype(bf),
            "WqfA0": np.ascontiguousarray(WqfA0).astype(bf),
            "WqfB0": np.ascontiguousarray(WqfB0).astype(bf),
            "WqfC": WqfC.astype(bf),
            "WqfD": WqfD.astype(bf),
            "WkfAB": np.ascontiguousarray(WkfAB).astype(bf),
            "WoutA": ((WoutA * W8SC).astype(f8) if OUT_FP8
                      else WoutA.astype(bf)),
            "mask_pack": mask_pack.astype(bf),
            "ident": ident.astype(bf),
        })
    return in_maps


_BUILD_CACHE = {}


def run_sharded(inputs, T=2048, trace=False):
    if T not in _BUILD_CACHE:
        _BUILD_CACHE[T] = build_program(T=T)
    nc = _BUILD_CACHE[T]
    in_maps = _host_prep(T=T, **inputs)
    res = run_bass_kernel_spmd(nc, in_maps, core_ids=list(range(8)), trace=trace)
    x = inputs["x"]
    B = x.shape[0]
    out = np.array(x[:, :T], np.float32, copy=True)
    for c in range(8):
        out[c // 4] += res.results[c]["out"].T.astype(np.float32)
    return out, res


def kernel(**inputs):
    inputs = {k: np.asarray(v, np.float32) for k, v in inputs.items()}
    out, _ = run_sharded(inputs, T=2048, trace=False)
    return out
